# revision 40
# baseline (speedup 1.0000x reference)
"""CrossFocusedLinearAttentionPrune kernel for 8x TRN2 NeuronCores.

Data-parallel over batch B=8: one batch element per core; the small CxC
weights / C-vectors are replicated (host pre-transposed + pre-cast).

Per-core pipeline (channel-major = [C on partitions, spatial on free]):
  1. load q/k/v row-major via gpsimd casting-DMA (fp32 HBM -> bf16 SBUF)
  2. PE-transpose 128x128 blocks (identity matmul) -> channel-major
  3. q/k projections (bf16 matmul) -> fused relu((x+eps)/sc) on ACT,
     square on ACT, cube via DVE scalar_tensor_tensor (+k_sum accum)
  4. v projection row-major (stationary = transposed v tiles)
  5. kv = k3^T @ v (contraction over M, k3 re-transposed to row-major)
  6. z = 1/(q3 . k_sum + eps), broadcast via K=1 ones-matmul
  7. x = (q3 @ kv) * z, evicted into a zero-padded 68x68 channel-major map
  8. depthwise 5x5 conv = 25 PSUM-accumulated diagonal matmuls per c-block
     (taps are free-dim AP offsets into the padded map)
  9. h = conv + dwc_b + q3;  out = h @ Wproj^T + bproj (row-major) -> DRAM
"""

import os

import numpy as np
import ml_dtypes

import concourse.bacc as bacc
import concourse.bass as bass
import concourse.mybir as mybir
import concourse.tile as tile
from concourse.bass_utils import run_bass_kernel_spmd

F32 = mybir.dt.float32
BF16 = mybir.dt.bfloat16
AF = mybir.ActivationFunctionType
ALU = mybir.AluOpType

B, N, C = 8, 4096, 256
H = W = 64
KS, PAD = 5, 2
HP = H + 2 * PAD  # 68
EPS = 1e-6
CT = 2            # channel tiles of 128
NCH = 8           # 512-wide chunks over N
CHUNK = 512
NT = 32           # 128-row tiles over N
BF16NP = ml_dtypes.bfloat16


def build_program():
    nc = bacc.Bacc("TRN2", target_bir_lowering=False, debug=False,
                   enable_asserts=False, num_devices=8)

    # -------- DRAM tensors (per-core inputs) --------
    q_in = nc.dram_tensor("q_in", [N, C], F32, kind="ExternalInput").ap()
    k_in = nc.dram_tensor("k_in", [N, C], F32, kind="ExternalInput").ap()
    v_in = nc.dram_tensor("v_in", [N, C], F32, kind="ExternalInput").ap()
    wqT = nc.dram_tensor("wqT", [C, C], BF16, kind="ExternalInput").ap()
    wkT = nc.dram_tensor("wkT", [C, C], BF16, kind="ExternalInput").ap()
    wvT = nc.dram_tensor("wvT", [C, C], BF16, kind="ExternalInput").ap()
    wpT = nc.dram_tensor("wpT", [C, C], BF16, kind="ExternalInput").ap()
    diag = nc.dram_tensor("diag", [CT * 25, 128, 128], BF16,
                          kind="ExternalInput").ap()
    ident = nc.dram_tensor("ident", [128, 128], BF16, kind="ExternalInput").ap()
    srcp = nc.dram_tensor("screcip", [CT, 128], F32, kind="ExternalInput").ap()
    epsc = nc.dram_tensor("epssc", [CT, 128], F32, kind="ExternalInput").ap()
    dwcb = nc.dram_tensor("dwcb", [CT, 128], F32, kind="ExternalInput").ap()
    bpb = nc.dram_tensor("bprojb", [128, C], F32, kind="ExternalInput").ap()
    out_d = nc.dram_tensor("out", [N, C], F32, kind="ExternalOutput").ap()

    q_r = q_in.rearrange("(nt p) c -> p nt c", p=128)
    k_r = k_in.rearrange("(nt p) c -> p nt c", p=128)
    v_r = v_in.rearrange("(nt p) c -> p nt c", p=128)
    out_r = out_d.rearrange("(nt p) c -> p nt c", p=128)

    with tile.TileContext(nc) as tc:
        with (
            tc.tile_pool(name="const", bufs=1) as const,
            tc.tile_pool(name="big", bufs=1) as big,
            tc.tile_pool(name="rmbf", bufs=NCH) as rmbf,
            tc.tile_pool(name="tb", bufs=2) as tb,
            tc.tile_pool(name="vtb", bufs=4) as vtb,
            tc.tile_pool(name="k3cp", bufs=3) as k3cp,
            tc.tile_pool(name="k3p", bufs=NT * CT) as k3p,
            tc.tile_pool(name="vrmp", bufs=3) as vrmp,
            tc.tile_pool(name="mp", bufs=2) as mp,
            tc.tile_pool(name="smal", bufs=1) as smal,
            tc.tile_pool(name="psA", bufs=2, space="PSUM") as psA,
            tc.tile_pool(name="psKV", bufs=1, space="PSUM") as psKV,
            tc.tile_pool(name="psB", bufs=2, space="PSUM") as psB,
            tc.tile_pool(name="psT", bufs=2, space="PSUM") as psT,
        ):
            # -------- constants into SBUF --------
            wq_sb = const.tile([128, CT, C], BF16)
            nc.sync.dma_start(wq_sb[:], wqT.rearrange("(ct p) d -> p ct d", p=128))
            wk_sb = const.tile([128, CT, C], BF16)
            nc.sync.dma_start(wk_sb[:], wkT.rearrange("(ct p) d -> p ct d", p=128))
            wv_sb = const.tile([128, CT, C], BF16)
            nc.sync.dma_start(wv_sb[:], wvT.rearrange("(ct p) d -> p ct d", p=128))
            wp_sb = const.tile([128, CT, C], BF16)
            nc.sync.dma_start(wp_sb[:], wpT.rearrange("(ct p) d -> p ct d", p=128))
            d_sb = const.tile([128, CT * 25, 128], BF16)
            nc.sync.dma_start(d_sb[:], diag.rearrange("t p m -> p t m"))
            id_sb = const.tile([128, 128], BF16)
            nc.sync.dma_start(id_sb[:], ident)
            sr_sb = const.tile([128, CT], F32)
            nc.sync.dma_start(sr_sb[:], srcp.rearrange("ct p -> p ct"))
            ep_sb = const.tile([128, CT], F32)
            nc.sync.dma_start(ep_sb[:], epsc.rearrange("ct p -> p ct"))
            db_sb = const.tile([128, CT], F32)
            nc.sync.dma_start(db_sb[:], dwcb.rearrange("ct p -> p ct"))
            bp_sb = const.tile([128, C], F32)
            nc.sync.dma_start(bp_sb[:], bpb)

            # -------- big persistent tensors --------
            q3 = big.tile([128, CT, N], BF16)           # focused q, channel-major
            h = big.tile([128, CT, N], BF16)            # conv-out + q3
            xpad = big.tile([128, CT, HP * HP], BF16)   # padded attention map
            k3blk = {}                                  # k3 row-major blocks
            ksum_p = smal.tile([128, CT * NCH], F32)    # per-chunk k3 row-sums
            ksum_bf = smal.tile([128, CT], BF16)
            z_linb = smal.tile([1, N], BF16)            # z_num staged as a row
            znr = smal.tile([128, NT], BF16)            # znr[p,f] = z_num[32p+f]
            znr2 = smal.tile([128, NT], F32)
            zrec = smal.tile([128, NT], F32)            # per-partition z scalars
            kv_sb = smal.tile([128, CT, C], BF16)
            out_stage = big.tile([128, NT, C], F32)

            nc.vector.memset(xpad[:], 0.0)
            xv = xpad.rearrange("p ct (r c) -> p ct r c", r=HP)

            def pe_transpose(dst_block, src_block):
                # dst[128,128] (SBUF bf16) = src[128,128].T via PE + ACT evict
                ps = psT.tile([128, 128], BF16, tag="t", name="tps")
                nc.tensor.transpose(ps[:], src_block, id_sb[:])
                nc.scalar.copy(dst_block, ps[:])

            # ================= Q phase =================
            for ch in range(NCH):
                qrm = rmbf.tile([128, 4, C], BF16, tag="qrm", name=f"qrm{ch}")
                nc.gpsimd.dma_start(qrm[:], q_r[:, 4 * ch:4 * ch + 4, :])
                qT = tb.tile([128, CT, CHUNK], BF16, tag="qt", name=f"qT{ch}")
                for ct in range(CT):
                    for g in range(4):
                        pe_transpose(qT[:, ct, g * 128:(g + 1) * 128],
                                     qrm[:, g, ct * 128:(ct + 1) * 128])
                for dt in range(CT):
                    qps = psA.tile([128, CHUNK], F32, tag="s")
                    for ct in range(CT):
                        nc.tensor.matmul(qps[:], lhsT=wq_sb[:, ct, dt * 128:(dt + 1) * 128],
                                         rhs=qT[:, ct, :], start=(ct == 0), stop=(ct == 1))
                    m = mp.tile([128, CHUNK], F32, tag="m")
                    nc.scalar.activation(m[:], qps[:], AF.Relu,
                                         bias=ep_sb[:, dt:dt + 1],
                                         scale=sr_sb[:, dt:dt + 1])
                    m2 = psB.tile([128, CHUNK], F32, tag="b")
                    nc.scalar.activation(m2[:], m[:], AF.Square)
                    nc.vector.scalar_tensor_tensor(
                        q3[:, dt, ch * CHUNK:(ch + 1) * CHUNK],
                        m2[:], 1.0, m[:], op0=ALU.bypass, op1=ALU.mult)

            # ================= K phase =================
            for ch in range(NCH):
                krm = rmbf.tile([128, 4, C], BF16, tag="krm", name=f"krm{ch}")
                nc.gpsimd.dma_start(krm[:], k_r[:, 4 * ch:4 * ch + 4, :])
                kT = tb.tile([128, CT, CHUNK], BF16, tag="kt", name=f"kT{ch}")
                for ct in range(CT):
                    for g in range(4):
                        pe_transpose(kT[:, ct, g * 128:(g + 1) * 128],
                                     krm[:, g, ct * 128:(ct + 1) * 128])
                for dt in range(CT):
                    kps = psA.tile([128, CHUNK], F32, tag="s")
                    for ct in range(CT):
                        nc.tensor.matmul(kps[:], lhsT=wk_sb[:, ct, dt * 128:(dt + 1) * 128],
                                         rhs=kT[:, ct, :], start=(ct == 0), stop=(ct == 1))
                    m = mp.tile([128, CHUNK], F32, tag="m")
                    nc.scalar.activation(m[:], kps[:], AF.Relu,
                                         bias=ep_sb[:, dt:dt + 1],
                                         scale=sr_sb[:, dt:dt + 1])
                    m2 = psB.tile([128, CHUNK], F32, tag="b")
                    nc.scalar.activation(m2[:], m[:], AF.Square)
                    k3c = k3cp.tile([128, CHUNK], BF16, tag="k3")
                    nc.vector.scalar_tensor_tensor(
                        k3c[:], m2[:], 1.0, m[:], op0=ALU.bypass, op1=ALU.mult,
                        accum_out=ksum_p[:, dt * NCH + ch:dt * NCH + ch + 1])
                    # k3 row-major blocks for the kv contraction
                    for g in range(4):
                        nt = 4 * ch + g
                        blk = k3p.tile([128, 128], BF16, tag="k3b",
                                       name=f"k3b{nt}_{dt}")
                        k3blk[(nt, dt)] = blk
                        pe_transpose(blk[:], k3c[:, g * 128:(g + 1) * 128])

            # ================= V + kv phase =================
            kv_ps = []
            for dt in range(CT):
                t = psKV.tile([128, C], F32, tag=f"kv{dt}", name=f"kvps{dt}")
                kv_ps.append(t)
            for ch in range(NCH):
                vrm = rmbf.tile([128, 4, C], BF16, tag="vrm", name=f"vrm{ch}")
                nc.gpsimd.dma_start(vrm[:], v_r[:, 4 * ch:4 * ch + 4, :])
                for g in range(4):
                    nt = 4 * ch + g
                    vT = vtb.tile([128, CT, 128], BF16, tag="vt", name=f"vT{nt}")
                    for ct in range(CT):
                        pe_transpose(vT[:, ct, :], vrm[:, g, ct * 128:(ct + 1) * 128])
                    vps = psA.tile([128, C], F32, tag="s")
                    for ct in range(CT):
                        nc.tensor.matmul(vps[:], lhsT=vT[:, ct, :], rhs=wv_sb[:, ct, :],
                                         start=(ct == 0), stop=(ct == 1))
                    vrmt = vrmp.tile([128, C], BF16, tag="vr")
                    nc.scalar.copy(vrmt[:], vps[:])
                    for dt in range(CT):
                        nc.tensor.matmul(kv_ps[dt][:], lhsT=k3blk[(nt, dt)][:],
                                         rhs=vrmt[:], start=(nt == 0), stop=(nt == NT - 1))

            # ================= k_sum, z =================
            ksum_f = smal.tile([128, CT], F32)
            for dt in range(CT):
                nc.vector.reduce_sum(ksum_f[:, dt:dt + 1],
                                     ksum_p[:, dt * NCH:(dt + 1) * NCH],
                                     axis=mybir.AxisListType.X)
            nc.vector.tensor_copy(ksum_bf[:], ksum_f[:])

            for ch in range(NCH):
                zps = psA.tile([1, CHUNK], F32, tag="s")
                for ct in range(CT):
                    nc.tensor.matmul(zps[:], lhsT=ksum_bf[:, ct:ct + 1],
                                     rhs=q3[:, ct, ch * CHUNK:(ch + 1) * CHUNK],
                                     start=(ct == 0), stop=(ct == 1))
                nc.scalar.copy(z_linb[0:1, ch * CHUNK:(ch + 1) * CHUNK], zps[:])
            # one scatter: [1,4096] -> [128,32]  (znr[p,f] = z_num[32p+f])
            nc.sync.dma_start(znr[:], z_linb[:])
            nc.vector.tensor_scalar_add(znr2[:], znr[:], EPS)
            nc.vector.reciprocal(zrec[:], znr2[:])

            # ===== kv evict, x phase (stride-32 interleaved row tiles) =====
            # x-tile f holds rows n = 32*j + f (j = partition), so z is the
            # per-partition scalar zrec[:, f].
            for dt in range(CT):
                nc.scalar.copy(kv_sb[:, dt, :], kv_ps[dt][:])
            q3i = q3.rearrange("p ct (j f) -> p ct f j", f=NT)
            for f in range(NT):
                xps = psA.tile([128, C], F32, tag="s")
                for ct in range(CT):
                    nc.tensor.matmul(xps[:], lhsT=q3i[:, ct, f, :],
                                     rhs=kv_sb[:, ct, :], start=(ct == 0), stop=(ct == 1))
                xsb = vrmp.tile([128, C], BF16, tag="xr", name=f"xr{f}")
                nc.vector.tensor_scalar(xsb[:], xps[:], zrec[:, f:f + 1], None,
                                        op0=ALU.mult)
                # transpose into the padded channel-major conv map:
                # psT col j=2a+b -> spatial n = 64a + 32b + f
                for dt in range(CT):
                    ps = psT.tile([128, 128], BF16, tag="t", name="xtps")
                    nc.tensor.transpose(ps[:], xsb[:, dt * 128:(dt + 1) * 128],
                                        id_sb[:])
                    nc.scalar.copy(
                        xv[:, dt, 2:2 + H, 2 + f:2 + f + 33:32],
                        ps.rearrange("p (a b) -> p a b", b=2))

            # ================= depthwise conv + h =================
            for dt in range(CT):
                for ch in range(NCH):
                    cps = psB.tile([128, CHUNK], F32, tag="b")
                    t = 0
                    for dy in range(-PAD, PAD + 1):
                        for dx in range(-PAD, PAD + 1):
                            rs = 8 * ch + 2 + dy
                            cs = 2 + dx
                            nc.tensor.matmul(
                                cps[:], lhsT=d_sb[:, dt * 25 + t, :],
                                rhs=xv[:, dt, rs:rs + 8, cs:cs + W],
                                start=(t == 0), stop=(t == 24))
                            t += 1
                    nc.vector.scalar_tensor_tensor(
                        h[:, dt, ch * CHUNK:(ch + 1) * CHUNK],
                        cps[:], db_sb[:, dt:dt + 1],
                        q3[:, dt, ch * CHUNK:(ch + 1) * CHUNK],
                        op0=ALU.add, op1=ALU.add)

            # ================= final projection =================
            for nt in range(NT):
                ops = psA.tile([128, C], F32, tag="s")
                for ct in range(CT):
                    nc.tensor.matmul(ops[:], lhsT=h[:, ct, nt * 128:(nt + 1) * 128],
                                     rhs=wp_sb[:, ct, :], start=(ct == 0), stop=(ct == 1))
                nc.vector.tensor_add(out_stage[:, nt, :], ops[:], bp_sb[:])
            # one store for the whole output (avoids coarse DRAM WAW waits)
            nc.sync.dma_start(out_r[:], out_stage[:])

    nc.compile()
    return nc


_CACHE = {}


def _get_nc():
    if "nc" not in _CACHE:
        _CACHE["nc"] = build_program()
    return _CACHE["nc"]


def _host_prep(Wq, Wk, Wv, Wproj, bproj, dwc_w, dwc_b, scale):
    sc = np.logaddexp(0.0, scale.reshape(C).astype(np.float64)).astype(np.float32)
    screcip = (1.0 / sc).reshape(CT, 128)
    epssc = (EPS / sc).reshape(CT, 128)
    diag = np.zeros((CT * 25, 128, 128), dtype=np.float32)
    w = dwc_w.reshape(C, KS * KS)
    for ct in range(CT):
        for t in range(25):
            np.fill_diagonal(diag[ct * 25 + t], w[ct * 128:(ct + 1) * 128, t])
    shared = {
        "wqT": np.ascontiguousarray(Wq.T).astype(BF16NP),
        "wkT": np.ascontiguousarray(Wk.T).astype(BF16NP),
        "wvT": np.ascontiguousarray(Wv.T).astype(BF16NP),
        "wpT": np.ascontiguousarray(Wproj.T).astype(BF16NP),
        "diag": diag.astype(BF16NP),
        "ident": np.eye(128, dtype=np.float32).astype(BF16NP),
        "screcip": screcip.astype(np.float32),
        "epssc": epssc.astype(np.float32),
        "dwcb": dwc_b.reshape(CT, 128).astype(np.float32),
        "bprojb": np.ascontiguousarray(
            np.broadcast_to(bproj.reshape(1, C), (128, C))).astype(np.float32),
    }
    return shared


def kernel(query, key, value, Wq, Wk, Wv, Wproj, bproj, dwc_w, dwc_b, scale,
           H=64, W=64, **_unused):
    assert int(H) == 64 and int(W) == 64
    query = np.asarray(query, dtype=np.float32)
    key = np.asarray(key, dtype=np.float32)
    value = np.asarray(value, dtype=np.float32)
    shared = _host_prep(np.asarray(Wq, np.float32), np.asarray(Wk, np.float32),
                        np.asarray(Wv, np.float32), np.asarray(Wproj, np.float32),
                        np.asarray(bproj, np.float32), np.asarray(dwc_w, np.float32),
                        np.asarray(dwc_b, np.float32), np.asarray(scale, np.float32))
    in_maps = []
    for b in range(B):
        m = dict(shared)
        m["q_in"] = np.ascontiguousarray(query[b])
        m["k_in"] = np.ascontiguousarray(key[b])
        m["v_in"] = np.ascontiguousarray(value[b])
        in_maps.append(m)
    nc = _get_nc()
    trace = os.environ.get("KERNEL_PROFILE") == "1"
    kw = {}
    if trace:
        kw["trace"] = True
        d = os.environ.get("KERNEL_PROFILE_DIR")
        if d:
            os.makedirs(d, exist_ok=True)
            kw["tmpdir"] = d
    try:
        res = run_bass_kernel_spmd(nc, in_maps, list(range(B)), **kw)
    except ModuleNotFoundError:
        # NTFF profile hook not available in this container; run untraced
        kw.pop("trace", None)
        kw.pop("tmpdir", None)
        res = run_bass_kernel_spmd(nc, in_maps, list(range(B)), **kw)
    _CACHE["last_res"] = res
    if trace and res.exec_time_ns is not None:
        print(f"HW exec time: {res.exec_time_ns} ns")
    out = np.stack([np.asarray(res.results[i]["out"], dtype=np.float32)
                    for i in range(B)])
    return out


# revision 44
# speedup vs baseline: 1.1504x; 1.1504x over previous
"""CrossFocusedLinearAttentionPrune kernel for 8x TRN2 NeuronCores.

Data-parallel over batch B=8: one batch element per core; the small CxC
weights / C-vectors are replicated (host pre-transposed + pre-cast).

Per-core pipeline (channel-major = [C on partitions, spatial on free]):
  1. load q/k/v row-major via gpsimd casting-DMA (fp32 HBM -> bf16 SBUF)
  2. PE-transpose 128x128 blocks (identity matmul) -> channel-major
  3. q/k projections (bf16 matmul) -> fused relu((x+eps)/sc) on ACT,
     square on ACT, cube via DVE scalar_tensor_tensor (+k_sum accum)
  4. v projection row-major (stationary = transposed v tiles)
  5. kv = k3^T @ v (contraction over M, k3 re-transposed to row-major)
  6. z = 1/(q3 . k_sum + eps), broadcast via K=1 ones-matmul
  7. x = (q3 @ kv) * z, evicted into a zero-padded 68x68 channel-major map
  8. depthwise 5x5 conv = 25 PSUM-accumulated diagonal matmuls per c-block
     (taps are free-dim AP offsets into the padded map)
  9. h = conv + dwc_b + q3;  out = h @ Wproj^T + bproj (row-major) -> DRAM
"""

import os

import numpy as np
import ml_dtypes

import concourse.bacc as bacc
import concourse.bass as bass
import concourse.mybir as mybir
import concourse.tile as tile
from concourse.bass_utils import run_bass_kernel_spmd

F32 = mybir.dt.float32
BF16 = mybir.dt.bfloat16
AF = mybir.ActivationFunctionType
ALU = mybir.AluOpType

B, N, C = 8, 4096, 256
H = W = 64
KS, PAD = 5, 2
HP = H + 2 * PAD  # 68
EPS = 1e-6
CT = 2            # channel tiles of 128
NCH = 8           # 512-wide chunks over N
CHUNK = 512
NT = 32           # 128-row tiles over N
BF16NP = ml_dtypes.bfloat16


def build_program():
    nc = bacc.Bacc("TRN2", target_bir_lowering=False, debug=False,
                   enable_asserts=False, num_devices=8)

    # -------- DRAM tensors (per-core inputs) --------
    q_in = nc.dram_tensor("q_in", [N, C], F32, kind="ExternalInput").ap()
    k_in = nc.dram_tensor("k_in", [N, C], F32, kind="ExternalInput").ap()
    v_in = nc.dram_tensor("v_in", [N, C], F32, kind="ExternalInput").ap()
    wqT = nc.dram_tensor("wqT", [C, C], BF16, kind="ExternalInput").ap()
    wkT = nc.dram_tensor("wkT", [C, C], BF16, kind="ExternalInput").ap()
    wvT = nc.dram_tensor("wvT", [C, C], BF16, kind="ExternalInput").ap()
    wpT = nc.dram_tensor("wpT", [C, C], BF16, kind="ExternalInput").ap()
    diag = nc.dram_tensor("diag", [CT * 25, 128, 128], BF16,
                          kind="ExternalInput").ap()
    ident = nc.dram_tensor("ident", [128, 128], BF16, kind="ExternalInput").ap()
    srcp = nc.dram_tensor("screcip", [CT, 128], F32, kind="ExternalInput").ap()
    epsc = nc.dram_tensor("epssc", [CT, 128], F32, kind="ExternalInput").ap()
    dwcb = nc.dram_tensor("dwcb", [CT, 128], F32, kind="ExternalInput").ap()
    bpb = nc.dram_tensor("bprojb", [128, C], F32, kind="ExternalInput").ap()
    out_d = nc.dram_tensor("out", [N, C], F32, kind="ExternalOutput").ap()

    q_r = q_in.rearrange("(nt p) c -> p nt c", p=128)
    k_r = k_in.rearrange("(nt p) c -> p nt c", p=128)
    v_r = v_in.rearrange("(nt p) c -> p nt c", p=128)
    out_r = out_d.rearrange("(nt p) c -> p nt c", p=128)

    with tile.TileContext(nc) as tc:
        with (
            tc.tile_pool(name="const", bufs=1) as const,
            tc.tile_pool(name="big", bufs=1) as big,
            tc.tile_pool(name="rmbf", bufs=NCH) as rmbf,
            tc.tile_pool(name="tb", bufs=2) as tb,
            tc.tile_pool(name="vtb", bufs=4) as vtb,
            tc.tile_pool(name="k3cp", bufs=3) as k3cp,
            tc.tile_pool(name="k3p", bufs=NT * CT) as k3p,
            tc.tile_pool(name="vrmp", bufs=3) as vrmp,
            tc.tile_pool(name="mp", bufs=2) as mp,
            tc.tile_pool(name="smal", bufs=1) as smal,
            tc.tile_pool(name="psA", bufs=3, space="PSUM") as psA,
            tc.tile_pool(name="psKV", bufs=1, space="PSUM") as psKV,
            tc.tile_pool(name="psB", bufs=2, space="PSUM") as psB,
            tc.tile_pool(name="psT", bufs=2, space="PSUM") as psT,
        ):
            # -------- constants into SBUF --------
            wq_sb = const.tile([128, CT, C], BF16)
            nc.sync.dma_start(wq_sb[:], wqT.rearrange("(ct p) d -> p ct d", p=128))
            wk_sb = const.tile([128, CT, C], BF16)
            nc.sync.dma_start(wk_sb[:], wkT.rearrange("(ct p) d -> p ct d", p=128))
            wv_sb = const.tile([128, CT, C], BF16)
            nc.sync.dma_start(wv_sb[:], wvT.rearrange("(ct p) d -> p ct d", p=128))
            wp_sb = const.tile([128, CT, C], BF16)
            nc.sync.dma_start(wp_sb[:], wpT.rearrange("(ct p) d -> p ct d", p=128))
            d_sb = const.tile([128, CT * 25, 128], BF16)
            nc.sync.dma_start(d_sb[:], diag.rearrange("t p m -> p t m"))
            id_sb = const.tile([128, 128], BF16)
            nc.sync.dma_start(id_sb[:], ident)
            sr_sb = const.tile([128, CT], F32)
            nc.sync.dma_start(sr_sb[:], srcp.rearrange("ct p -> p ct"))
            ep_sb = const.tile([128, CT], F32)
            nc.sync.dma_start(ep_sb[:], epsc.rearrange("ct p -> p ct"))
            db_sb = const.tile([128, CT], F32)
            nc.sync.dma_start(db_sb[:], dwcb.rearrange("ct p -> p ct"))
            bp_sb = const.tile([128, C], F32)
            nc.sync.dma_start(bp_sb[:], bpb)

            # -------- big persistent tensors --------
            q3 = big.tile([128, CT, N], BF16)           # focused q, channel-major
            h = big.tile([128, CT, N], BF16)            # conv-out + q3
            xpad = big.tile([128, CT, HP * HP], BF16)   # padded attention map
            k3blk = {}                                  # k3 row-major blocks
            ksum_p = smal.tile([128, CT * NCH], F32)    # per-chunk k3 row-sums
            ksum_bf = smal.tile([128, CT], BF16)
            z_linb = smal.tile([1, N], BF16)            # z_num staged as a row
            znr = smal.tile([128, NT], BF16)            # znr[p,f] = z_num[32p+f]
            znr2 = smal.tile([128, NT], F32)
            zrec = smal.tile([128, NT], F32)            # per-partition z scalars
            kv_sb = smal.tile([128, CT, C], BF16)
            out_stage = big.tile([128, NT, C], F32)

            nc.vector.memset(xpad[:], 0.0)
            xv = xpad.rearrange("p ct (r c) -> p ct r c", r=HP)

            def pe_transpose(dst_block, src_block):
                # dst[128,128] (SBUF bf16) = src[128,128].T via PE + DVE evict
                ps = psT.tile([128, 128], BF16, tag="t", name="tps")
                nc.tensor.transpose(ps[:], src_block, id_sb[:])
                nc.vector.tensor_copy(dst_block, ps[:])

            # ================= Q phase =================
            for ch in range(NCH):
                qrm = rmbf.tile([128, 4, C], BF16, tag="qrm", name=f"qrm{ch}")
                nc.gpsimd.dma_start(qrm[:], q_r[:, 4 * ch:4 * ch + 4, :])
                qT = tb.tile([128, CT, CHUNK], BF16, tag="qt", name=f"qT{ch}")
                for ct in range(CT):
                    for g in range(4):
                        pe_transpose(qT[:, ct, g * 128:(g + 1) * 128],
                                     qrm[:, g, ct * 128:(ct + 1) * 128])
                for dt in range(CT):
                    qps = psA.tile([128, CHUNK], F32, tag="s")
                    for ct in range(CT):
                        nc.tensor.matmul(qps[:], lhsT=wq_sb[:, ct, dt * 128:(dt + 1) * 128],
                                         rhs=qT[:, ct, :], start=(ct == 0), stop=(ct == 1))
                    m = mp.tile([128, CHUNK], F32, tag="m")
                    nc.scalar.activation(m[:], qps[:], AF.Relu,
                                         bias=ep_sb[:, dt:dt + 1],
                                         scale=sr_sb[:, dt:dt + 1])
                    m2 = psB.tile([128, CHUNK], F32, tag="b")
                    nc.scalar.activation(m2[:], m[:], AF.Square)
                    nc.vector.scalar_tensor_tensor(
                        q3[:, dt, ch * CHUNK:(ch + 1) * CHUNK],
                        m2[:], 1.0, m[:], op0=ALU.bypass, op1=ALU.mult)

            # ================= K phase =================
            for ch in range(NCH):
                krm = rmbf.tile([128, 4, C], BF16, tag="krm", name=f"krm{ch}")
                nc.gpsimd.dma_start(krm[:], k_r[:, 4 * ch:4 * ch + 4, :])
                kT = tb.tile([128, CT, CHUNK], BF16, tag="kt", name=f"kT{ch}")
                for ct in range(CT):
                    for g in range(4):
                        pe_transpose(kT[:, ct, g * 128:(g + 1) * 128],
                                     krm[:, g, ct * 128:(ct + 1) * 128])
                for dt in range(CT):
                    kps = psA.tile([128, CHUNK], F32, tag="s")
                    for ct in range(CT):
                        nc.tensor.matmul(kps[:], lhsT=wk_sb[:, ct, dt * 128:(dt + 1) * 128],
                                         rhs=kT[:, ct, :], start=(ct == 0), stop=(ct == 1))
                    m = mp.tile([128, CHUNK], F32, tag="m")
                    nc.scalar.activation(m[:], kps[:], AF.Relu,
                                         bias=ep_sb[:, dt:dt + 1],
                                         scale=sr_sb[:, dt:dt + 1])
                    m2 = psB.tile([128, CHUNK], F32, tag="b")
                    nc.scalar.activation(m2[:], m[:], AF.Square)
                    k3c = k3cp.tile([128, CHUNK], BF16, tag="k3")
                    nc.vector.scalar_tensor_tensor(
                        k3c[:], m2[:], 1.0, m[:], op0=ALU.bypass, op1=ALU.mult,
                        accum_out=ksum_p[:, dt * NCH + ch:dt * NCH + ch + 1])
                    # k3 row-major blocks for the kv contraction
                    for g in range(4):
                        nt = 4 * ch + g
                        blk = k3p.tile([128, 128], BF16, tag="k3b",
                                       name=f"k3b{nt}_{dt}")
                        k3blk[(nt, dt)] = blk
                        pe_transpose(blk[:], k3c[:, g * 128:(g + 1) * 128])

            # ================= V + kv phase =================
            kv_one = psKV.tile([128, 2 * C], F32, tag="kv", name="kvps")
            kv_ps = [kv_one[:, 0:C], kv_one[:, C:2 * C]]
            for ch in range(NCH):
                vrm = rmbf.tile([128, 4, C], BF16, tag="vrm", name=f"vrm{ch}")
                nc.gpsimd.dma_start(vrm[:], v_r[:, 4 * ch:4 * ch + 4, :])
                for g in range(4):
                    nt = 4 * ch + g
                    vT = vtb.tile([128, CT, 128], BF16, tag="vt", name=f"vT{nt}")
                    for ct in range(CT):
                        pe_transpose(vT[:, ct, :], vrm[:, g, ct * 128:(ct + 1) * 128])
                    vps = psA.tile([128, C], F32, tag="s")
                    for ct in range(CT):
                        nc.tensor.matmul(vps[:], lhsT=vT[:, ct, :], rhs=wv_sb[:, ct, :],
                                         start=(ct == 0), stop=(ct == 1))
                    vrmt = vrmp.tile([128, C], BF16, tag="vr")
                    nc.scalar.copy(vrmt[:], vps[:])
                    for dt in range(CT):
                        nc.tensor.matmul(kv_ps[dt][:], lhsT=k3blk[(nt, dt)][:],
                                         rhs=vrmt[:], start=(nt == 0), stop=(nt == NT - 1))

            # ================= k_sum, z =================
            ksum_f = smal.tile([128, CT], F32)
            for dt in range(CT):
                nc.vector.reduce_sum(ksum_f[:, dt:dt + 1],
                                     ksum_p[:, dt * NCH:(dt + 1) * NCH],
                                     axis=mybir.AxisListType.X)
            nc.vector.tensor_copy(ksum_bf[:], ksum_f[:])

            for ch in range(NCH):
                zps = psA.tile([1, CHUNK], F32, tag="s")
                for ct in range(CT):
                    nc.tensor.matmul(zps[:], lhsT=ksum_bf[:, ct:ct + 1],
                                     rhs=q3[:, ct, ch * CHUNK:(ch + 1) * CHUNK],
                                     start=(ct == 0), stop=(ct == 1))
                nc.scalar.copy(z_linb[0:1, ch * CHUNK:(ch + 1) * CHUNK], zps[:])
            # one scatter: [1,4096] -> [128,32]  (znr[p,f] = z_num[32p+f])
            nc.sync.dma_start(znr[:], z_linb[:])
            nc.vector.tensor_scalar_add(znr2[:], znr[:], EPS)
            nc.vector.reciprocal(zrec[:], znr2[:])

            # ===== kv evict, x phase (stride-32 interleaved row tiles) =====
            # x-tile f holds rows n = 32*j + f (j = partition), so z is the
            # per-partition scalar zrec[:, f].
            for dt in range(CT):
                nc.scalar.copy(kv_sb[:, dt, :], kv_ps[dt][:])
            q3i = q3.rearrange("p ct (j f) -> p ct f j", f=NT)
            for f in range(NT):
                xps = psA.tile([128, C], F32, tag="s")
                for ct in range(CT):
                    nc.tensor.matmul(xps[:], lhsT=q3i[:, ct, f, :],
                                     rhs=kv_sb[:, ct, :], start=(ct == 0), stop=(ct == 1))
                xsb = vrmp.tile([128, C], BF16, tag="xr", name=f"xr{f}")
                nc.vector.tensor_scalar(xsb[:], xps[:], zrec[:, f:f + 1], None,
                                        op0=ALU.mult)
                # transpose into the padded channel-major conv map:
                # psT col j=2a+b -> spatial n = 64a + 32b + f
                for dt in range(CT):
                    ps = psT.tile([128, 128], BF16, tag="t", name="xtps")
                    nc.tensor.transpose(ps[:], xsb[:, dt * 128:(dt + 1) * 128],
                                        id_sb[:])
                    nc.scalar.copy(
                        xv[:, dt, 2:2 + H, 2 + f:2 + f + 33:32],
                        ps.rearrange("p (a b) -> p a b", b=2))

            # ================= depthwise conv + h =================
            for dt in range(CT):
                for ch in range(NCH):
                    cps = psB.tile([128, CHUNK], F32, tag="b")
                    t = 0
                    for dy in range(-PAD, PAD + 1):
                        for dx in range(-PAD, PAD + 1):
                            rs = 8 * ch + 2 + dy
                            cs = 2 + dx
                            nc.tensor.matmul(
                                cps[:], lhsT=d_sb[:, dt * 25 + t, :],
                                rhs=xv[:, dt, rs:rs + 8, cs:cs + W],
                                start=(t == 0), stop=(t == 24))
                            t += 1
                    nc.vector.scalar_tensor_tensor(
                        h[:, dt, ch * CHUNK:(ch + 1) * CHUNK],
                        cps[:], db_sb[:, dt:dt + 1],
                        q3[:, dt, ch * CHUNK:(ch + 1) * CHUNK],
                        op0=ALU.add, op1=ALU.add)

            # ================= final projection =================
            for nt in range(NT):
                ops = psA.tile([128, C], F32, tag="s")
                for ct in range(CT):
                    nc.tensor.matmul(ops[:], lhsT=h[:, ct, nt * 128:(nt + 1) * 128],
                                     rhs=wp_sb[:, ct, :], start=(ct == 0), stop=(ct == 1))
                nc.vector.tensor_add(out_stage[:, nt, :], ops[:], bp_sb[:])
            # one store for the whole output (avoids coarse DRAM WAW waits)
            nc.sync.dma_start(out_r[:], out_stage[:])

    nc.compile()
    return nc


_CACHE = {}


def _get_nc():
    if "nc" not in _CACHE:
        _CACHE["nc"] = build_program()
    return _CACHE["nc"]


def _host_prep(Wq, Wk, Wv, Wproj, bproj, dwc_w, dwc_b, scale):
    sc = np.logaddexp(0.0, scale.reshape(C).astype(np.float64)).astype(np.float32)
    screcip = (1.0 / sc).reshape(CT, 128)
    epssc = (EPS / sc).reshape(CT, 128)
    diag = np.zeros((CT * 25, 128, 128), dtype=np.float32)
    w = dwc_w.reshape(C, KS * KS)
    for ct in range(CT):
        for t in range(25):
            np.fill_diagonal(diag[ct * 25 + t], w[ct * 128:(ct + 1) * 128, t])
    shared = {
        "wqT": np.ascontiguousarray(Wq.T).astype(BF16NP),
        "wkT": np.ascontiguousarray(Wk.T).astype(BF16NP),
        "wvT": np.ascontiguousarray(Wv.T).astype(BF16NP),
        "wpT": np.ascontiguousarray(Wproj.T).astype(BF16NP),
        "diag": diag.astype(BF16NP),
        "ident": np.eye(128, dtype=np.float32).astype(BF16NP),
        "screcip": screcip.astype(np.float32),
        "epssc": epssc.astype(np.float32),
        "dwcb": dwc_b.reshape(CT, 128).astype(np.float32),
        "bprojb": np.ascontiguousarray(
            np.broadcast_to(bproj.reshape(1, C), (128, C))).astype(np.float32),
    }
    return shared


def kernel(query, key, value, Wq, Wk, Wv, Wproj, bproj, dwc_w, dwc_b, scale,
           H=64, W=64, **_unused):
    assert int(H) == 64 and int(W) == 64
    query = np.asarray(query, dtype=np.float32)
    key = np.asarray(key, dtype=np.float32)
    value = np.asarray(value, dtype=np.float32)
    shared = _host_prep(np.asarray(Wq, np.float32), np.asarray(Wk, np.float32),
                        np.asarray(Wv, np.float32), np.asarray(Wproj, np.float32),
                        np.asarray(bproj, np.float32), np.asarray(dwc_w, np.float32),
                        np.asarray(dwc_b, np.float32), np.asarray(scale, np.float32))
    in_maps = []
    for b in range(B):
        m = dict(shared)
        m["q_in"] = np.ascontiguousarray(query[b])
        m["k_in"] = np.ascontiguousarray(key[b])
        m["v_in"] = np.ascontiguousarray(value[b])
        in_maps.append(m)
    nc = _get_nc()
    trace = os.environ.get("KERNEL_PROFILE") == "1"
    kw = {}
    if trace:
        kw["trace"] = True
        d = os.environ.get("KERNEL_PROFILE_DIR")
        if d:
            os.makedirs(d, exist_ok=True)
            kw["tmpdir"] = d
    try:
        res = run_bass_kernel_spmd(nc, in_maps, list(range(B)), **kw)
    except ModuleNotFoundError:
        # NTFF profile hook not available in this container; run untraced
        kw.pop("trace", None)
        kw.pop("tmpdir", None)
        res = run_bass_kernel_spmd(nc, in_maps, list(range(B)), **kw)
    _CACHE["last_res"] = res
    if trace and res.exec_time_ns is not None:
        print(f"HW exec time: {res.exec_time_ns} ns")
    out = np.stack([np.asarray(res.results[i]["out"], dtype=np.float32)
                    for i in range(B)])
    return out


# revision 45
# speedup vs baseline: 1.1548x; 1.0039x over previous
"""CrossFocusedLinearAttentionPrune kernel for 8x TRN2 NeuronCores.

Data-parallel over batch B=8: one batch element per core; the small CxC
weights / C-vectors are replicated (host pre-transposed + pre-cast).

Per-core pipeline (channel-major = [C on partitions, spatial on free]):
  1. load q/k/v row-major via gpsimd casting-DMA (fp32 HBM -> bf16 SBUF)
  2. PE-transpose 128x128 blocks (identity matmul) -> channel-major
  3. q/k projections (bf16 matmul) -> fused relu((x+eps)/sc) on ACT,
     square on ACT, cube via DVE scalar_tensor_tensor (+k_sum accum)
  4. v projection row-major (stationary = transposed v tiles)
  5. kv = k3^T @ v (contraction over M, k3 re-transposed to row-major)
  6. z = 1/(q3 . k_sum + eps), broadcast via K=1 ones-matmul
  7. x = (q3 @ kv) * z, evicted into a zero-padded 68x68 channel-major map
  8. depthwise 5x5 conv = 25 PSUM-accumulated diagonal matmuls per c-block
     (taps are free-dim AP offsets into the padded map)
  9. h = conv + dwc_b + q3;  out = h @ Wproj^T + bproj (row-major) -> DRAM
"""

import os

import numpy as np
import ml_dtypes

import concourse.bacc as bacc
import concourse.bass as bass
import concourse.mybir as mybir
import concourse.tile as tile
from concourse.bass_utils import run_bass_kernel_spmd

F32 = mybir.dt.float32
BF16 = mybir.dt.bfloat16
AF = mybir.ActivationFunctionType
ALU = mybir.AluOpType

B, N, C = 8, 4096, 256
H = W = 64
KS, PAD = 5, 2
HP = H + 2 * PAD  # 68
EPS = 1e-6
CT = 2            # channel tiles of 128
NCH = 8           # 512-wide chunks over N
CHUNK = 512
NT = 32           # 128-row tiles over N
BF16NP = ml_dtypes.bfloat16


def build_program():
    nc = bacc.Bacc("TRN2", target_bir_lowering=False, debug=False,
                   enable_asserts=False, num_devices=8)

    # -------- DRAM tensors (per-core inputs) --------
    q_in = nc.dram_tensor("q_in", [N, C], F32, kind="ExternalInput").ap()
    k_in = nc.dram_tensor("k_in", [N, C], F32, kind="ExternalInput").ap()
    v_in = nc.dram_tensor("v_in", [N, C], F32, kind="ExternalInput").ap()
    wqT = nc.dram_tensor("wqT", [C, C], BF16, kind="ExternalInput").ap()
    wkT = nc.dram_tensor("wkT", [C, C], BF16, kind="ExternalInput").ap()
    wvT = nc.dram_tensor("wvT", [C, C], BF16, kind="ExternalInput").ap()
    wpT = nc.dram_tensor("wpT", [C, C], BF16, kind="ExternalInput").ap()
    diag = nc.dram_tensor("diag", [CT * 25, 128, 128], BF16,
                          kind="ExternalInput").ap()
    ident = nc.dram_tensor("ident", [128, 128], BF16, kind="ExternalInput").ap()
    srcp = nc.dram_tensor("screcip", [CT, 128], F32, kind="ExternalInput").ap()
    epsc = nc.dram_tensor("epssc", [CT, 128], F32, kind="ExternalInput").ap()
    dwcb = nc.dram_tensor("dwcb", [CT, 128], F32, kind="ExternalInput").ap()
    bpb = nc.dram_tensor("bprojb", [128, C], F32, kind="ExternalInput").ap()
    out_d = nc.dram_tensor("out", [N, C], F32, kind="ExternalOutput").ap()

    q_r = q_in.rearrange("(nt p) c -> p nt c", p=128)
    k_r = k_in.rearrange("(nt p) c -> p nt c", p=128)
    v_r = v_in.rearrange("(nt p) c -> p nt c", p=128)
    out_r = out_d.rearrange("(nt p) c -> p nt c", p=128)

    with tile.TileContext(nc) as tc:
        with (
            tc.tile_pool(name="const", bufs=1) as const,
            tc.tile_pool(name="big", bufs=1) as big,
            tc.tile_pool(name="rmbf", bufs=NCH) as rmbf,
            tc.tile_pool(name="tb", bufs=3) as tb,
            tc.tile_pool(name="vtb", bufs=6) as vtb,
            tc.tile_pool(name="k3cp", bufs=4) as k3cp,
            tc.tile_pool(name="k3p", bufs=NT * CT) as k3p,
            tc.tile_pool(name="vrmp", bufs=4) as vrmp,
            tc.tile_pool(name="mp", bufs=3) as mp,
            tc.tile_pool(name="smal", bufs=1) as smal,
            tc.tile_pool(name="psA", bufs=3, space="PSUM") as psA,
            tc.tile_pool(name="psKV", bufs=1, space="PSUM") as psKV,
            tc.tile_pool(name="psB", bufs=2, space="PSUM") as psB,
            tc.tile_pool(name="psT", bufs=2, space="PSUM") as psT,
        ):
            # -------- constants into SBUF --------
            wq_sb = const.tile([128, CT, C], BF16)
            nc.sync.dma_start(wq_sb[:], wqT.rearrange("(ct p) d -> p ct d", p=128))
            wk_sb = const.tile([128, CT, C], BF16)
            nc.sync.dma_start(wk_sb[:], wkT.rearrange("(ct p) d -> p ct d", p=128))
            wv_sb = const.tile([128, CT, C], BF16)
            nc.sync.dma_start(wv_sb[:], wvT.rearrange("(ct p) d -> p ct d", p=128))
            wp_sb = const.tile([128, CT, C], BF16)
            nc.sync.dma_start(wp_sb[:], wpT.rearrange("(ct p) d -> p ct d", p=128))
            d_sb = const.tile([128, CT * 25, 128], BF16)
            nc.sync.dma_start(d_sb[:], diag.rearrange("t p m -> p t m"))
            id_sb = const.tile([128, 128], BF16)
            nc.sync.dma_start(id_sb[:], ident)
            sr_sb = const.tile([128, CT], F32)
            nc.sync.dma_start(sr_sb[:], srcp.rearrange("ct p -> p ct"))
            ep_sb = const.tile([128, CT], F32)
            nc.sync.dma_start(ep_sb[:], epsc.rearrange("ct p -> p ct"))
            db_sb = const.tile([128, CT], F32)
            nc.sync.dma_start(db_sb[:], dwcb.rearrange("ct p -> p ct"))
            bp_sb = const.tile([128, C], F32)
            nc.sync.dma_start(bp_sb[:], bpb)

            # -------- big persistent tensors --------
            q3 = big.tile([128, CT, N], BF16)           # focused q, channel-major
            h = big.tile([128, CT, N], BF16)            # conv-out + q3
            xpad = big.tile([128, CT, HP * HP], BF16)   # padded attention map
            k3blk = {}                                  # k3 row-major blocks
            ksum_p = smal.tile([128, CT * NCH], F32)    # per-chunk k3 row-sums
            ksum_bf = smal.tile([128, CT], BF16)
            z_linb = smal.tile([1, N], BF16)            # z_num staged as a row
            znr = smal.tile([128, NT], BF16)            # znr[p,f] = z_num[32p+f]
            znr2 = smal.tile([128, NT], F32)
            zrec = smal.tile([128, NT], F32)            # per-partition z scalars
            kv_sb = smal.tile([128, CT, C], BF16)
            out_stage = big.tile([128, NT, C], F32)

            nc.vector.memset(xpad[:], 0.0)
            xv = xpad.rearrange("p ct (r c) -> p ct r c", r=HP)

            def pe_transpose(dst_block, src_block):
                # dst[128,128] (SBUF bf16) = src[128,128].T via PE + DVE evict
                ps = psT.tile([128, 128], BF16, tag="t", name="tps")
                nc.tensor.transpose(ps[:], src_block, id_sb[:])
                nc.vector.tensor_copy(dst_block, ps[:])

            # ================= Q phase =================
            for ch in range(NCH):
                qrm = rmbf.tile([128, 4, C], BF16, tag="qrm", name=f"qrm{ch}")
                nc.gpsimd.dma_start(qrm[:], q_r[:, 4 * ch:4 * ch + 4, :])
                qT = tb.tile([128, CT, CHUNK], BF16, tag="qt", name=f"qT{ch}")
                for ct in range(CT):
                    for g in range(4):
                        pe_transpose(qT[:, ct, g * 128:(g + 1) * 128],
                                     qrm[:, g, ct * 128:(ct + 1) * 128])
                for dt in range(CT):
                    qps = psA.tile([128, CHUNK], F32, tag="s")
                    for ct in range(CT):
                        nc.tensor.matmul(qps[:], lhsT=wq_sb[:, ct, dt * 128:(dt + 1) * 128],
                                         rhs=qT[:, ct, :], start=(ct == 0), stop=(ct == 1))
                    m = mp.tile([128, CHUNK], F32, tag="m")
                    nc.scalar.activation(m[:], qps[:], AF.Relu,
                                         bias=ep_sb[:, dt:dt + 1],
                                         scale=sr_sb[:, dt:dt + 1])
                    m2 = psB.tile([128, CHUNK], F32, tag="b")
                    nc.scalar.activation(m2[:], m[:], AF.Square)
                    nc.vector.scalar_tensor_tensor(
                        q3[:, dt, ch * CHUNK:(ch + 1) * CHUNK],
                        m2[:], 1.0, m[:], op0=ALU.bypass, op1=ALU.mult)

            # ================= K phase =================
            for ch in range(NCH):
                krm = rmbf.tile([128, 4, C], BF16, tag="krm", name=f"krm{ch}")
                nc.gpsimd.dma_start(krm[:], k_r[:, 4 * ch:4 * ch + 4, :])
                kT = tb.tile([128, CT, CHUNK], BF16, tag="kt", name=f"kT{ch}")
                for ct in range(CT):
                    for g in range(4):
                        pe_transpose(kT[:, ct, g * 128:(g + 1) * 128],
                                     krm[:, g, ct * 128:(ct + 1) * 128])
                for dt in range(CT):
                    kps = psA.tile([128, CHUNK], F32, tag="s")
                    for ct in range(CT):
                        nc.tensor.matmul(kps[:], lhsT=wk_sb[:, ct, dt * 128:(dt + 1) * 128],
                                         rhs=kT[:, ct, :], start=(ct == 0), stop=(ct == 1))
                    m = mp.tile([128, CHUNK], F32, tag="m")
                    nc.scalar.activation(m[:], kps[:], AF.Relu,
                                         bias=ep_sb[:, dt:dt + 1],
                                         scale=sr_sb[:, dt:dt + 1])
                    m2 = psB.tile([128, CHUNK], F32, tag="b")
                    nc.scalar.activation(m2[:], m[:], AF.Square)
                    k3c = k3cp.tile([128, CHUNK], BF16, tag="k3")
                    nc.vector.scalar_tensor_tensor(
                        k3c[:], m2[:], 1.0, m[:], op0=ALU.bypass, op1=ALU.mult,
                        accum_out=ksum_p[:, dt * NCH + ch:dt * NCH + ch + 1])
                    # k3 row-major blocks for the kv contraction
                    for g in range(4):
                        nt = 4 * ch + g
                        blk = k3p.tile([128, 128], BF16, tag="k3b",
                                       name=f"k3b{nt}_{dt}")
                        k3blk[(nt, dt)] = blk
                        pe_transpose(blk[:], k3c[:, g * 128:(g + 1) * 128])

            # ================= V + kv phase =================
            kv_one = psKV.tile([128, 2 * C], F32, tag="kv", name="kvps")
            kv_ps = [kv_one[:, 0:C], kv_one[:, C:2 * C]]
            for ch in range(NCH):
                vrm = rmbf.tile([128, 4, C], BF16, tag="vrm", name=f"vrm{ch}")
                nc.gpsimd.dma_start(vrm[:], v_r[:, 4 * ch:4 * ch + 4, :])
                for g in range(4):
                    nt = 4 * ch + g
                    vT = vtb.tile([128, CT, 128], BF16, tag="vt", name=f"vT{nt}")
                    for ct in range(CT):
                        pe_transpose(vT[:, ct, :], vrm[:, g, ct * 128:(ct + 1) * 128])
                    vps = psA.tile([128, C], F32, tag="s")
                    for ct in range(CT):
                        nc.tensor.matmul(vps[:], lhsT=vT[:, ct, :], rhs=wv_sb[:, ct, :],
                                         start=(ct == 0), stop=(ct == 1))
                    vrmt = vrmp.tile([128, C], BF16, tag="vr")
                    nc.scalar.copy(vrmt[:], vps[:])
                    for dt in range(CT):
                        nc.tensor.matmul(kv_ps[dt][:], lhsT=k3blk[(nt, dt)][:],
                                         rhs=vrmt[:], start=(nt == 0), stop=(nt == NT - 1))

            # ================= k_sum, z =================
            ksum_f = smal.tile([128, CT], F32)
            for dt in range(CT):
                nc.vector.reduce_sum(ksum_f[:, dt:dt + 1],
                                     ksum_p[:, dt * NCH:(dt + 1) * NCH],
                                     axis=mybir.AxisListType.X)
            nc.vector.tensor_copy(ksum_bf[:], ksum_f[:])

            for ch in range(NCH):
                zps = psA.tile([1, CHUNK], F32, tag="s")
                for ct in range(CT):
                    nc.tensor.matmul(zps[:], lhsT=ksum_bf[:, ct:ct + 1],
                                     rhs=q3[:, ct, ch * CHUNK:(ch + 1) * CHUNK],
                                     start=(ct == 0), stop=(ct == 1))
                nc.scalar.copy(z_linb[0:1, ch * CHUNK:(ch + 1) * CHUNK], zps[:])
            # one scatter: [1,4096] -> [128,32]  (znr[p,f] = z_num[32p+f])
            nc.sync.dma_start(znr[:], z_linb[:])
            nc.vector.tensor_scalar_add(znr2[:], znr[:], EPS)
            nc.vector.reciprocal(zrec[:], znr2[:])

            # ===== kv evict, x phase (stride-32 interleaved row tiles) =====
            # x-tile f holds rows n = 32*j + f (j = partition), so z is the
            # per-partition scalar zrec[:, f].
            for dt in range(CT):
                nc.scalar.copy(kv_sb[:, dt, :], kv_ps[dt][:])
            q3i = q3.rearrange("p ct (j f) -> p ct f j", f=NT)
            for f in range(NT):
                xps = psA.tile([128, C], F32, tag="s")
                for ct in range(CT):
                    nc.tensor.matmul(xps[:], lhsT=q3i[:, ct, f, :],
                                     rhs=kv_sb[:, ct, :], start=(ct == 0), stop=(ct == 1))
                xsb = vrmp.tile([128, C], BF16, tag="xr", name=f"xr{f}")
                nc.vector.tensor_scalar(xsb[:], xps[:], zrec[:, f:f + 1], None,
                                        op0=ALU.mult)
                # transpose into the padded channel-major conv map:
                # psT col j=2a+b -> spatial n = 64a + 32b + f
                for dt in range(CT):
                    ps = psT.tile([128, 128], BF16, tag="t", name="xtps")
                    nc.tensor.transpose(ps[:], xsb[:, dt * 128:(dt + 1) * 128],
                                        id_sb[:])
                    nc.scalar.copy(
                        xv[:, dt, 2:2 + H, 2 + f:2 + f + 33:32],
                        ps.rearrange("p (a b) -> p a b", b=2))

            # ================= depthwise conv + h =================
            for dt in range(CT):
                for ch in range(NCH):
                    cps = psB.tile([128, CHUNK], F32, tag="b")
                    t = 0
                    for dy in range(-PAD, PAD + 1):
                        for dx in range(-PAD, PAD + 1):
                            rs = 8 * ch + 2 + dy
                            cs = 2 + dx
                            nc.tensor.matmul(
                                cps[:], lhsT=d_sb[:, dt * 25 + t, :],
                                rhs=xv[:, dt, rs:rs + 8, cs:cs + W],
                                start=(t == 0), stop=(t == 24))
                            t += 1
                    nc.vector.scalar_tensor_tensor(
                        h[:, dt, ch * CHUNK:(ch + 1) * CHUNK],
                        cps[:], db_sb[:, dt:dt + 1],
                        q3[:, dt, ch * CHUNK:(ch + 1) * CHUNK],
                        op0=ALU.add, op1=ALU.add)

            # ================= final projection =================
            for nt in range(NT):
                ops = psA.tile([128, C], F32, tag="s")
                for ct in range(CT):
                    nc.tensor.matmul(ops[:], lhsT=h[:, ct, nt * 128:(nt + 1) * 128],
                                     rhs=wp_sb[:, ct, :], start=(ct == 0), stop=(ct == 1))
                nc.vector.tensor_add(out_stage[:, nt, :], ops[:], bp_sb[:])
            # one store for the whole output (avoids coarse DRAM WAW waits)
            nc.sync.dma_start(out_r[:], out_stage[:])

    nc.compile()
    return nc


_CACHE = {}


def _get_nc():
    if "nc" not in _CACHE:
        _CACHE["nc"] = build_program()
    return _CACHE["nc"]


def _host_prep(Wq, Wk, Wv, Wproj, bproj, dwc_w, dwc_b, scale):
    sc = np.logaddexp(0.0, scale.reshape(C).astype(np.float64)).astype(np.float32)
    screcip = (1.0 / sc).reshape(CT, 128)
    epssc = (EPS / sc).reshape(CT, 128)
    diag = np.zeros((CT * 25, 128, 128), dtype=np.float32)
    w = dwc_w.reshape(C, KS * KS)
    for ct in range(CT):
        for t in range(25):
            np.fill_diagonal(diag[ct * 25 + t], w[ct * 128:(ct + 1) * 128, t])
    shared = {
        "wqT": np.ascontiguousarray(Wq.T).astype(BF16NP),
        "wkT": np.ascontiguousarray(Wk.T).astype(BF16NP),
        "wvT": np.ascontiguousarray(Wv.T).astype(BF16NP),
        "wpT": np.ascontiguousarray(Wproj.T).astype(BF16NP),
        "diag": diag.astype(BF16NP),
        "ident": np.eye(128, dtype=np.float32).astype(BF16NP),
        "screcip": screcip.astype(np.float32),
        "epssc": epssc.astype(np.float32),
        "dwcb": dwc_b.reshape(CT, 128).astype(np.float32),
        "bprojb": np.ascontiguousarray(
            np.broadcast_to(bproj.reshape(1, C), (128, C))).astype(np.float32),
    }
    return shared


def kernel(query, key, value, Wq, Wk, Wv, Wproj, bproj, dwc_w, dwc_b, scale,
           H=64, W=64, **_unused):
    assert int(H) == 64 and int(W) == 64
    query = np.asarray(query, dtype=np.float32)
    key = np.asarray(key, dtype=np.float32)
    value = np.asarray(value, dtype=np.float32)
    shared = _host_prep(np.asarray(Wq, np.float32), np.asarray(Wk, np.float32),
                        np.asarray(Wv, np.float32), np.asarray(Wproj, np.float32),
                        np.asarray(bproj, np.float32), np.asarray(dwc_w, np.float32),
                        np.asarray(dwc_b, np.float32), np.asarray(scale, np.float32))
    in_maps = []
    for b in range(B):
        m = dict(shared)
        m["q_in"] = np.ascontiguousarray(query[b])
        m["k_in"] = np.ascontiguousarray(key[b])
        m["v_in"] = np.ascontiguousarray(value[b])
        in_maps.append(m)
    nc = _get_nc()
    trace = os.environ.get("KERNEL_PROFILE") == "1"
    kw = {}
    if trace:
        kw["trace"] = True
        d = os.environ.get("KERNEL_PROFILE_DIR")
        if d:
            os.makedirs(d, exist_ok=True)
            kw["tmpdir"] = d
    try:
        res = run_bass_kernel_spmd(nc, in_maps, list(range(B)), **kw)
    except ModuleNotFoundError:
        # NTFF profile hook not available in this container; run untraced
        kw.pop("trace", None)
        kw.pop("tmpdir", None)
        res = run_bass_kernel_spmd(nc, in_maps, list(range(B)), **kw)
    _CACHE["last_res"] = res
    if trace and res.exec_time_ns is not None:
        print(f"HW exec time: {res.exec_time_ns} ns")
    out = np.stack([np.asarray(res.results[i]["out"], dtype=np.float32)
                    for i in range(B)])
    return out
